# revision 18
# baseline (speedup 1.0000x reference)
"""Trainium2 Bass kernel for nn_CrossAttention (XCA-style cross channel-attention).

Sharding: 8 cores = (batch b, branch). branch 0 computes ctx_out[b] (q from x,
k/v from ctx); branch 1 computes x_out[b] (q from ctx, k/v from x). Each core:
  conv1x1 qkv (PE) -> depthwise 3x3 (PE, diagonal-weight matmuls with shifted
  access patterns + PSUM accumulation) -> l2norm (ACT square-accum + small ops)
  -> per-head channel Gram (PE, transposed layout) -> softmax (DVE/ACT) ->
  attn@v (PE) -> conv1x1 po (PE) -> DMA out.
Compute in bf16 with fp32 PSUM accumulation; norms/softmax in fp32.
"""

import os
import numpy as np
import ml_dtypes

import concourse.bass as bass
import concourse.tile as tile
from concourse import bacc, mybir
from concourse.bass_utils import run_bass_kernel_spmd

BF = ml_dtypes.bfloat16
F32 = mybir.dt.float32
BF16 = mybir.dt.bfloat16

DIM = 192
H = W = 128
N = H * W
NH, HD = 4, 48
P0, P1 = 128, 64
RB = 4              # image rows per block
WB = RB * W         # 512 free elems per block
NB = H // RB        # 32 blocks
EPS = 1e-12

CFG = {"wa": 3, "rng": 4, "dsb": 2, "scrp": 2, "psc": 2, "pst": 2,
       "rngB": 4, "dsbB": 2, "osb": 2, "pcv": 2,
       "dwq2": 1, "dwk2": 1, "dwm2": 1}

AF = mybir.ActivationFunctionType
AX = mybir.AxisListType
OP = mybir.AluOpType


def _dw_segments(j):
    """Matmul segments for 3x3 depthwise conv of block j (rows j*RB..j*RB+RB).

    Returns list of (tap, src_block, y0, y1, r0, r1, sx0, sx1, ox0, ox1):
    out[:, y0:y1, ox0:ox1] += diag(w_tap) @ src[src_block][:, r0:r1, sx0:sx1]
    Center tap (full coverage) is first.
    """
    segs = []
    for ky in range(3):
        for kx in range(3):
            dy, dx = ky - 1, kx - 1
            tt = ky * 3 + kx
            ox0, ox1 = max(0, -dx), W - max(0, dx)
            sx0, sx1 = ox0 + dx, ox1 + dx
            ylo = max(j * RB, max(0, -dy))
            yhi = min(j * RB + RB, H - max(0, dy))
            Y = ylo
            while Y < yhi:
                jj = (Y + dy) // RB
                yend = min(yhi, (jj + 1) * RB - dy)
                r0 = (Y + dy) % RB
                if dx == 0:
                    # full-width rows are contiguous: one matmul per segment
                    segs.append((tt, jj, Y - j * RB, yend - j * RB,
                                 r0, r0 + (yend - Y), sx0, sx1, ox0, ox1))
                else:
                    # shifted columns: per-row matmuls (sim/HW want flat free APs)
                    for yy in range(Y, yend):
                        rr = r0 + (yy - Y)
                        segs.append((tt, jj, yy - j * RB, yy - j * RB + 1,
                                     rr, rr + 1, sx0, sx1, ox0, ox1))
                Y = yend
    segs.sort(key=lambda s: 0 if s[0] == 4 else 1)
    return segs


def _emit_dw(nc, pd, ring, dg, j):
    segs = _dw_segments(j)
    for idx, (tt, jj, y0, y1, r0, r1, sx0, sx1, ox0, ox1) in enumerate(segs):
        nc.tensor.matmul(
            pd[:, y0:y1, ox0:ox1],
            dg[:, tt, :],
            ring[jj][:, r0:r1, sx0:sx1],
            start=(idx == 0),
            stop=(idx == len(segs) - 1),
        )


def _emit(tc, t):
    nc = tc.nc
    fp, bf = F32, BF16

    with tc.tile_pool(name="cst", bufs=1) as cst:
        # ---- residents
        xb0 = cst.tile([P0, N], bf, tag="xb0")
        nc.sync.dma_start(out=xb0, in_=t["xb"][0:P0, :])
        xb1 = cst.tile([P1, N], bf, tag="xb1")
        nc.sync.dma_start(out=xb1, in_=t["xb"][P0:DIM, :])

        def wload(name, rows0, rows1):
            w0 = cst.tile([P0, DIM], bf, tag=name + "0")
            nc.sync.dma_start(out=w0, in_=t[name][0:P0, :])
            w1 = cst.tile([P1, DIM], bf, tag=name + "1")
            nc.sync.dma_start(out=w1, in_=t[name][P0:DIM, :])
            return w0, w1

        wq0, wq1 = wload("wq", P0, P1)
        wk0, wk1 = wload("wk", P0, P1)
        wv0, wv1 = wload("wv", P0, P1)
        wpoA = cst.tile([96, DIM], bf, tag="wpoA")
        nc.sync.dma_start(out=wpoA, in_=t["wpo"][0:96, :])
        wpoB = cst.tile([96, DIM], bf, tag="wpoB")
        nc.sync.dma_start(out=wpoB, in_=t["wpo"][96:DIM, :])

        def dload(name, p):
            d = cst.tile([p, 9, p], bf, tag=name)
            nc.sync.dma_start(out=d, in_=t[name])
            return d

        dq0 = dload("dq0", P0)
        dk0 = dload("dk0", P0)
        dqk1 = dload("dqk1", P0)
        dv0 = dload("dv0", 96)
        dv1 = dload("dv1", 96)
        idn = cst.tile([P0, P0], bf, tag="idn")
        nc.sync.dma_start(out=idn, in_=t["ident"])
        tsc0 = cst.tile([P0, 1], fp, tag="tsc0")
        nc.sync.dma_start(out=tsc0, in_=t["tsc"][0:P0, :])
        tsc1 = cst.tile([P1, 1], fp, tag="tsc1")
        nc.sync.dma_start(out=tsc1, in_=t["tsc"][P0:DIM, :])

        qn_p0 = cst.tile([P0, NB], fp, tag="qn_p0")
        kn_p0 = cst.tile([P0, NB], fp, tag="kn_p0")
        qkn_p1 = cst.tile([P0, NB], fp, tag="qkn_p1")
        AT01 = cst.tile([96, 96], bf, tag="AT01")
        AT23 = cst.tile([96, 96], bf, tag="AT23")

        # ================= PASS A: q/k conv+dw+transpose, Gram, norms
        ring_q0, ring_k0, ring_m1 = [], [], []
        with tc.tile_pool(name="wa", bufs=CFG["wa"]) as wa, \
             tc.tile_pool(name="rng", bufs=CFG["rng"]) as rng, \
             tc.tile_pool(name="dsb", bufs=CFG["dsb"]) as dsb, \
             tc.tile_pool(name="scrp", bufs=CFG["scrp"]) as scrp, \
             tc.tile_pool(name="psc", bufs=CFG["psc"], space="PSUM") as psc, \
             tc.tile_pool(name="psd", bufs=1, space="PSUM") as psd, \
             tc.tile_pool(name="pst", bufs=CFG["pst"], space="PSUM") as pst, \
             tc.tile_pool(name="psg", bufs=1, space="PSUM") as psg:

            # head-pair Gram blocks GT[d, pair, c] (transposed logits):
            # pair 0 = k/q ch 0..95 (heads 0,1), pair 1 = ch 96..191 (heads 2,3)
            GT = psg.tile([96, 2, 96], fp, tag="GT")

            for i in range(NB + 1):
                if i < NB:
                    cols = slice(i * WB, (i + 1) * WB)
                    xa0 = wa.tile([P0, RB, W], bf, tag="xa0")
                    nc.sync.dma_start(out=xa0, in_=t["xa"][0:P0, cols])
                    xa1 = wa.tile([P1, RB, W], bf, tag="xa1")
                    nc.sync.dma_start(out=xa1, in_=t["xa"][P0:DIM, cols])

                    pq = psc.tile([P0, RB, W], fp, tag="conv")
                    nc.tensor.matmul(pq, wq0[:, 0:P0], xa0, start=True, stop=False)
                    nc.tensor.matmul(pq, wq1[:, 0:P0], xa1, start=False, stop=True)
                    q0 = rng.tile([P0, RB, W], bf, tag="q0")
                    ring_q0.append(q0)
                    nc.vector.tensor_copy(out=q0, in_=pq)

                    pk = psc.tile([P0, RB, W], fp, tag="conv")
                    nc.tensor.matmul(pk, wk0[:, 0:P0], xb0[:, cols], start=True, stop=False)
                    nc.tensor.matmul(pk, wk1[:, 0:P0], xb1[:, cols], start=False, stop=True)
                    k0 = rng.tile([P0, RB, W], bf, tag="k0")
                    ring_k0.append(k0)
                    nc.vector.tensor_copy(out=k0, in_=pk)

                    # merged chunk1: partitions 0:64 = q ch 128..191, 64:128 = k ch 128..191
                    pm = psc.tile([P0, RB, W], fp, tag="conv")
                    nc.tensor.matmul(pm[0:P1], wq0[:, P0:DIM], xa0, start=True, stop=False)
                    nc.tensor.matmul(pm[0:P1], wq1[:, P0:DIM], xa1, start=False, stop=True)
                    nc.tensor.matmul(pm[P1:P0], wk0[:, P0:DIM], xb0[:, cols], start=True, stop=False)
                    nc.tensor.matmul(pm[P1:P0], wk1[:, P0:DIM], xb1[:, cols], start=False, stop=True)
                    m1 = rng.tile([P0, RB, W], bf, tag="m1")
                    ring_m1.append(m1)
                    nc.vector.tensor_copy(out=m1, in_=pm)

                if i >= 1:
                    j = i - 1
                    pdq = psd.tile([P0, RB, W], fp, tag="dwq", bufs=CFG["dwq2"])
                    _emit_dw(nc, pdq, ring_q0, dq0, j)
                    pdk = psd.tile([P0, RB, W], fp, tag="dwk", bufs=CFG["dwk2"])
                    _emit_dw(nc, pdk, ring_k0, dk0, j)
                    pdm = psd.tile([P0, RB, W], fp, tag="dwm", bufs=CFG["dwm2"])
                    _emit_dw(nc, pdm, ring_m1, dqk1, j)

                    qdw = dsb.tile([P0, RB, W], bf, tag="qdw")
                    nc.vector.tensor_copy(out=qdw, in_=pdq)
                    kdw = dsb.tile([P0, RB, W], bf, tag="kdw")
                    nc.vector.tensor_copy(out=kdw, in_=pdk)
                    mdw = dsb.tile([P0, RB, W], bf, tag="mdw")
                    nc.vector.tensor_copy(out=mdw, in_=pdm)

                    scr = scrp.tile([P0, RB, W], fp, tag="scr")
                    nc.scalar.activation(out=scr, in_=pdq, func=AF.Square,
                                         accum_out=qn_p0[:, j:j + 1])
                    scr = scrp.tile([P0, RB, W], fp, tag="scr")
                    nc.scalar.activation(out=scr, in_=pdk, func=AF.Square,
                                         accum_out=kn_p0[:, j:j + 1])
                    scr = scrp.tile([P0, RB, W], fp, tag="scr")
                    nc.scalar.activation(out=scr, in_=pdm, func=AF.Square,
                                         accum_out=qkn_p1[:, j:j + 1])

                    qT = dsb.tile([P0, RB, DIM], bf, tag="qT")
                    kT = dsb.tile([P0, RB, DIM], bf, tag="kT")
                    for y in range(RB):
                        tp = pst.tile([P0, P0], bf, tag="tp")
                        nc.tensor.transpose(tp, qdw[:, y, :], idn)
                        nc.scalar.copy(out=qT[:, y, 0:P0], in_=tp)
                        tp = pst.tile([P0, P0], bf, tag="tp")
                        nc.tensor.transpose(tp, kdw[:, y, :], idn)
                        nc.scalar.copy(out=kT[:, y, 0:P0], in_=tp)
                        tp = pst.tile([P0, P0], bf, tag="tp")
                        nc.tensor.transpose(tp, mdw[:, y, :], idn)
                        nc.scalar.copy(out=qT[:, y, P0:DIM], in_=tp[:, 0:P1])
                        nc.scalar.copy(out=kT[:, y, P0:DIM], in_=tp[:, P1:P0])

                        for p in range(2):
                            ps_ = slice(96 * p, 96 * p + 96)
                            nc.tensor.matmul(GT[:, p, :], kT[:, y, ps_], qT[:, y, ps_],
                                             start=(j == 0 and y == 0 and p == 0),
                                             stop=(j == NB - 1 and y == RB - 1 and p == 1),
                                             skip_group_check=True)

            # ============ interlude: norms, logits scaling, softmax, A^T
            def rnorm(src, nm):
                d = cst.tile(list(src.shape[:1]) + [1], fp, tag=nm, name=nm)
                nc.scalar.activation(out=d, in_=src, func=AF.Sqrt)
                nc.vector.tensor_scalar_max(out=d, in0=d, scalar1=EPS)
                nc.vector.reciprocal(out=d, in_=d)
                return d

            qn2_0 = cst.tile([P0, 1], fp, tag="qn2_0")
            nc.vector.tensor_reduce(out=qn2_0, in_=qn_p0, axis=AX.X, op=OP.add)
            kn2_0 = cst.tile([P0, 1], fp, tag="kn2_0")
            nc.vector.tensor_reduce(out=kn2_0, in_=kn_p0, axis=AX.X, op=OP.add)
            mn2 = cst.tile([P0, 1], fp, tag="mn2")
            nc.vector.tensor_reduce(out=mn2, in_=qkn_p1, axis=AX.X, op=OP.add)

            rq0 = rnorm(qn2_0, "rq0")                       # [128,1] 1/||q|| ch 0..127
            rk0 = rnorm(kn2_0, "rk0")
            rm = rnorm(mn2, "rm")                          # [0:64]=q ch128.., [64:128]=k ch128..
            nc.vector.tensor_mul(out=rq0, in0=rq0, in1=tsc0)   # fold temperature
            nc.vector.tensor_mul(out=rm[0:P1], in0=rm[0:P1], in1=tsc1)

            # per-pair scale vectors at partition base 0 (cross-partition -> SWDGE DMA)
            # q channels: 0..127 in rq0, 128..191 in rm[0:64]
            # k channels: 0..127 in rk0, 128..191 in rm[64:128]
            rq_hi = cst.tile([96, 1], fp, tag="rq_hi")
            rk_hi = cst.tile([96, 1], fp, tag="rk_hi")
            nc.gpsimd.dma_start(out=rq_hi[0:32], in_=rq0[96:P0])
            nc.gpsimd.dma_start(out=rq_hi[32:96], in_=rm[0:P1])
            nc.gpsimd.dma_start(out=rk_hi[0:32], in_=rk0[96:P0])
            nc.gpsimd.dma_start(out=rk_hi[32:96], in_=rm[P1:P0])

            for p, (rkx, rqx, ATdst) in enumerate(
                    ((rk0[0:96], rq0[0:96], AT01), (rk_hi, rq_hi, AT23))):
                GTsb = cst.tile([96, 96], bf, tag=f"GTsb{p}", name=f"GTsb{p}")
                nc.vector.tensor_scalar_mul(out=GTsb, in0=GT[:, p, :], scalar1=rkx)
                Gp = pst.tile([P0, P0], bf, tag="tp")
                nc.tensor.transpose(Gp[0:96, 0:96], GTsb, idn[0:96, 0:96])
                Gsb = cst.tile([96, 96], fp, tag=f"Gsb{p}", name=f"Gsb{p}")
                nc.vector.tensor_scalar_mul(out=Gsb, in0=Gp[0:96, 0:96], scalar1=rqx)
                # head 1 of the pair sits at partition base 48 (not 32-aligned):
                # extract its [48,48] block to base 0 via SWDGE
                G1 = cst.tile([HD, HD], fp, tag=f"G1{p}", name=f"G1{p}")
                nc.gpsimd.dma_start(out=G1, in_=Gsb[HD:96, HD:96])

                nc.vector.memset(ATdst, 0.0)
                for hh, blk in ((0, Gsb[0:HD, 0:HD]), (1, G1)):
                    mneg = cst.tile([HD, 1], fp, tag=f"mneg{p}{hh}", name=f"mneg{p}{hh}")
                    nc.vector.tensor_reduce(out=mneg, in_=blk, axis=AX.X,
                                            op=OP.max, negate=True)
                    E = cst.tile([HD, HD], fp, tag=f"E{p}{hh}", name=f"E{p}{hh}")
                    nc.scalar.activation(out=E, in_=blk, func=AF.Exp,
                                         bias=mneg, scale=1.0)
                    sm = cst.tile([HD, 1], fp, tag=f"sm{p}{hh}", name=f"sm{p}{hh}")
                    nc.vector.tensor_reduce(out=sm, in_=E, axis=AX.X, op=OP.add)
                    rs = cst.tile([HD, 1], fp, tag=f"rs{p}{hh}", name=f"rs{p}{hh}")
                    nc.vector.reciprocal(out=rs, in_=sm)
                    Ah = cst.tile([HD, HD], bf, tag=f"Ah{p}{hh}", name=f"Ah{p}{hh}")
                    nc.vector.tensor_scalar_mul(out=Ah, in0=E, scalar1=rs)
                    ATp = pst.tile([P0, P0], bf, tag="tp")
                    nc.tensor.transpose(ATp[0:HD, 0:HD], Ah, idn[0:HD, 0:HD])
                    if hh == 0:
                        nc.scalar.copy(out=ATdst[0:HD, 0:HD], in_=ATp[0:HD, 0:HD])
                    else:
                        tmp = cst.tile([HD, HD], bf, tag=f"ATt{p}", name=f"ATt{p}")
                        nc.scalar.copy(out=tmp, in_=ATp[0:HD, 0:HD])
                        nc.gpsimd.dma_start(out=ATdst[HD:96, HD:96], in_=tmp)

        # ================= PASS B: v conv+dw, attn@v, po, out
        ring_v0, ring_v1 = [], []
        with tc.tile_pool(name="rngB", bufs=CFG["rngB"]) as rngB, \
             tc.tile_pool(name="dsbB", bufs=CFG["dsbB"]) as dsbB, \
             tc.tile_pool(name="osb", bufs=CFG["osb"]) as osb, \
             tc.tile_pool(name="pcv", bufs=CFG["pcv"], space="PSUM") as pcv, \
             tc.tile_pool(name="pdv", bufs=1, space="PSUM") as pdv, \
             tc.tile_pool(name="pav", bufs=1, space="PSUM") as pav, \
             tc.tile_pool(name="ppo", bufs=1, space="PSUM") as ppo:
            for i in range(NB + 1):
                if i < NB:
                    cols = slice(i * WB, (i + 1) * WB)
                    pv0 = pcv.tile([96, RB, W], fp, tag="cv")
                    nc.tensor.matmul(pv0, wv0[:, 0:96], xb0[:, cols], start=True, stop=False)
                    nc.tensor.matmul(pv0, wv1[:, 0:96], xb1[:, cols], start=False, stop=True)
                    v0 = rngB.tile([96, RB, W], bf, tag="v0")
                    ring_v0.append(v0)
                    nc.vector.tensor_copy(out=v0, in_=pv0)
                    pv1 = pcv.tile([96, RB, W], fp, tag="cv")
                    nc.tensor.matmul(pv1, wv0[:, 96:DIM], xb0[:, cols], start=True, stop=False)
                    nc.tensor.matmul(pv1, wv1[:, 96:DIM], xb1[:, cols], start=False, stop=True)
                    v1 = rngB.tile([96, RB, W], bf, tag="v1")
                    ring_v1.append(v1)
                    nc.vector.tensor_copy(out=v1, in_=pv1)
                if i >= 1:
                    j = i - 1
                    colj = slice(j * WB, (j + 1) * WB)
                    pd0 = pdv.tile([96, RB, W], fp, tag="dv0")
                    _emit_dw(nc, pd0, ring_v0, dv0, j)
                    vd0 = dsbB.tile([96, RB, W], bf, tag="vd0")
                    nc.vector.tensor_copy(out=vd0, in_=pd0)
                    pd1 = pdv.tile([96, RB, W], fp, tag="dv1")
                    _emit_dw(nc, pd1, ring_v1, dv1, j)
                    vd1 = dsbB.tile([96, RB, W], bf, tag="vd1")
                    nc.vector.tensor_copy(out=vd1, in_=pd1)

                    pa1 = pav.tile([96, RB, W], fp, tag="av01")
                    nc.tensor.matmul(pa1, AT01, vd0, start=True, stop=True)
                    pa2 = pav.tile([96, RB, W], fp, tag="av23")
                    nc.tensor.matmul(pa2, AT23, vd1, start=True, stop=True)
                    a1 = dsbB.tile([96, RB, W], bf, tag="a1")
                    nc.vector.tensor_copy(out=a1, in_=pa1)
                    a2 = dsbB.tile([96, RB, W], bf, tag="a2")
                    nc.vector.tensor_copy(out=a2, in_=pa2)

                    pp0 = ppo.tile([P0, RB, W], fp, tag="po0")
                    nc.tensor.matmul(pp0, wpoA[:, 0:P0], a1, start=True, stop=False)
                    nc.tensor.matmul(pp0, wpoB[:, 0:P0], a2, start=False, stop=True)
                    pp1 = ppo.tile([P1, RB, W], fp, tag="po1")
                    nc.tensor.matmul(pp1, wpoA[:, P0:DIM], a1, start=True, stop=False)
                    nc.tensor.matmul(pp1, wpoB[:, P0:DIM], a2, start=False, stop=True)
                    o0 = osb.tile([P0, RB, W], fp, tag="o0")
                    nc.vector.tensor_copy(out=o0, in_=pp0)
                    o1 = osb.tile([P1, RB, W], fp, tag="o1")
                    nc.vector.tensor_copy(out=o1, in_=pp1)
                    nc.sync.dma_start(out=t["out"][0:P0, colj], in_=o0)
                    nc.sync.dma_start(out=t["out"][P0:DIM, colj], in_=o1)


def _build():
    nc = bacc.Bacc("TRN2", target_bir_lowering=False, debug=False, num_devices=8)
    t = {}

    def din(name, shape, dt):
        t[name] = nc.dram_tensor(name, shape, dt, kind="ExternalInput").ap()

    din("xa", [DIM, N], BF16)
    din("xb", [DIM, N], BF16)
    din("wq", [DIM, DIM], BF16)
    din("wk", [DIM, DIM], BF16)
    din("wv", [DIM, DIM], BF16)
    din("wpo", [DIM, DIM], BF16)
    din("dq0", [P0, 9, P0], BF16)
    din("dk0", [P0, 9, P0], BF16)
    din("dqk1", [P0, 9, P0], BF16)
    din("dv0", [96, 9, 96], BF16)
    din("dv1", [96, 9, 96], BF16)
    din("tsc", [DIM, 1], F32)
    din("ident", [P0, P0], BF16)
    t["out"] = nc.dram_tensor("out", [DIM, N], F32, kind="ExternalOutput").ap()

    with tile.TileContext(nc) as tc:
        _emit(tc, t)
    nc.compile()
    return nc


def _diag_pack(w, p):
    """w [p, 9] -> [p, 9, p] with m[c, t, c] = w[c, t]."""
    m = np.zeros((p, 9, p), np.float32)
    i = np.arange(p)
    m[i, :, i] = w
    return m.astype(BF)


def _prep_core(inp, c):
    b, br = divmod(c, 2)
    x = np.asarray(inp["x"], np.float32)
    cx = np.asarray(inp["ctx"], np.float32)
    if br == 0:  # ctx branch -> ctx_out[b]: q from x, k/v from ctx
        A, Bm = x[b], cx[b]
        wqkv_a, dw_a = inp["x_qkv_w"], inp["x_dw_w"]
        wqkv_b, dw_b = inp["ctx_qkv_w"], inp["ctx_dw_w"]
        po = inp["ctx_po_w"]
    else:        # x branch -> x_out[b]: q from ctx, k/v from x
        A, Bm = cx[b], x[b]
        wqkv_a, dw_a = inp["ctx_qkv_w"], inp["ctx_dw_w"]
        wqkv_b, dw_b = inp["x_qkv_w"], inp["x_dw_w"]
        po = inp["x_po_w"]
    wqkv_a = np.asarray(wqkv_a, np.float32)[:, :, 0, 0]
    wqkv_b = np.asarray(wqkv_b, np.float32)[:, :, 0, 0]
    dw_a = np.asarray(dw_a, np.float32)[:, 0].reshape(3 * DIM, 9)
    dw_b = np.asarray(dw_b, np.float32)[:, 0].reshape(3 * DIM, 9)
    po = np.asarray(po, np.float32)[:, :, 0, 0]
    temp = np.asarray(inp["temperature"], np.float32).reshape(NH)

    dq = dw_a[0:DIM]
    dk = dw_b[DIM:2 * DIM]
    dv = dw_b[2 * DIM:3 * DIM]
    m1 = np.zeros((P0, 9, P0), np.float32)
    i64 = np.arange(P1)
    m1[i64, :, i64] = dq[P0:DIM]
    m1[i64 + P1, :, i64 + P1] = dk[P0:DIM]

    return {
        "xa": np.ascontiguousarray(A.reshape(DIM, N)).astype(BF),
        "xb": np.ascontiguousarray(Bm.reshape(DIM, N)).astype(BF),
        "wq": np.ascontiguousarray(wqkv_a[0:DIM].T).astype(BF),
        "wk": np.ascontiguousarray(wqkv_b[DIM:2 * DIM].T).astype(BF),
        "wv": np.ascontiguousarray(wqkv_b[2 * DIM:3 * DIM].T).astype(BF),
        "wpo": np.ascontiguousarray(po.T).astype(BF),
        "dq0": _diag_pack(dq[0:P0], P0),
        "dk0": _diag_pack(dk[0:P0], P0),
        "dqk1": m1.astype(BF),
        "dv0": _diag_pack(dv[0:96], 96),
        "dv1": _diag_pack(dv[96:DIM], 96),
        "tsc": np.repeat(temp, HD)[:, None].astype(np.float32),
        "ident": np.eye(P0, dtype=np.float32).astype(BF),
    }


_BUILT = None
LAST_RESULTS = None


def kernel(**inputs):
    global _BUILT, LAST_RESULTS
    if _BUILT is None:
        _BUILT = _build()
    in_maps = [_prep_core(inputs, c) for c in range(8)]
    res = run_bass_kernel_spmd(
        _BUILT, in_maps, list(range(8)),
        trace=bool(int(os.environ.get("KTRACE", "0"))),
    )
    LAST_RESULTS = res
    outs = [np.asarray(r["out"], np.float32) for r in res.results]
    x_out = np.stack([outs[2 * b + 1].reshape(DIM, H, W) for b in range(4)])
    ctx_out = np.stack([outs[2 * b].reshape(DIM, H, W) for b in range(4)])
    return (x_out, ctx_out)


# revision 25
# speedup vs baseline: 212.6681x; 212.6681x over previous
"""Trainium2 Bass kernel for nn_CrossAttention (XCA-style cross channel-attention).

Sharding: 8 cores = (batch b, branch). branch 0 computes ctx_out[b] (q from x,
k/v from ctx); branch 1 computes x_out[b] (q from ctx, k/v from x). Each core:
  conv1x1 qkv (PE) -> depthwise 3x3 (PE, diagonal-weight matmuls with shifted
  access patterns + PSUM accumulation) -> l2norm (ACT square-accum + small ops)
  -> per-head channel Gram (PE, transposed layout) -> softmax (DVE/ACT) ->
  attn@v (PE) -> conv1x1 po (PE) -> DMA out.
Compute in bf16 with fp32 PSUM accumulation; norms/softmax in fp32.
"""

import os
import numpy as np
import ml_dtypes

import concourse.bass as bass
import concourse.tile as tile
from concourse import bacc, mybir
from concourse.bass_utils import run_bass_kernel_spmd

BF = ml_dtypes.bfloat16
F32 = mybir.dt.float32
BF16 = mybir.dt.bfloat16

DIM = 192
H = W = 128
N = H * W
NH, HD = 4, 48
P0, P1 = 128, 64
RB = 4              # image rows per block
WB = RB * W         # 512 free elems per block
NB = H // RB        # 32 blocks
EPS = 1e-12

CFG = {"wa": 3, "rng": 4, "dsb": 2, "scrp": 2, "psc": 2, "pst": 2,
       "rngB": 4, "dsbB": 3, "osb": 2, "pcv": 2,
       "dwq2": 1, "dwk2": 1, "dwm2": 1}

AF = mybir.ActivationFunctionType
AX = mybir.AxisListType
OP = mybir.AluOpType


def _dw_segments(j):
    """Matmul segments for 3x3 depthwise conv of block j (rows j*RB..j*RB+RB).

    Returns list of (tap, src_block, y0, y1, r0, r1, sx0, sx1, ox0, ox1):
    out[:, y0:y1, ox0:ox1] += diag(w_tap) @ src[src_block][:, r0:r1, sx0:sx1]
    Center tap (full coverage) is first.
    """
    segs = []
    for ky in range(3):
        for kx in range(3):
            dy, dx = ky - 1, kx - 1
            tt = ky * 3 + kx
            ox0, ox1 = max(0, -dx), W - max(0, dx)
            sx0, sx1 = ox0 + dx, ox1 + dx
            ylo = max(j * RB, max(0, -dy))
            yhi = min(j * RB + RB, H - max(0, dy))
            Y = ylo
            while Y < yhi:
                jj = (Y + dy) // RB
                yend = min(yhi, (jj + 1) * RB - dy)
                r0 = (Y + dy) % RB
                if dx == 0:
                    # full-width rows are contiguous: one matmul per segment
                    segs.append((tt, jj, Y - j * RB, yend - j * RB,
                                 r0, r0 + (yend - Y), sx0, sx1, ox0, ox1))
                else:
                    # shifted columns: per-row matmuls (sim/HW want flat free APs)
                    for yy in range(Y, yend):
                        rr = r0 + (yy - Y)
                        segs.append((tt, jj, yy - j * RB, yy - j * RB + 1,
                                     rr, rr + 1, sx0, sx1, ox0, ox1))
                Y = yend
    segs.sort(key=lambda s: 0 if s[0] == 4 else 1)
    return segs


def _emit_dw(nc, pd, ring, dg, j):
    segs = _dw_segments(j)
    for idx, (tt, jj, y0, y1, r0, r1, sx0, sx1, ox0, ox1) in enumerate(segs):
        nc.tensor.matmul(
            pd[:, y0:y1, ox0:ox1],
            dg[:, tt, :],
            ring[jj][:, r0:r1, sx0:sx1],
            start=(idx == 0),
            stop=(idx == len(segs) - 1),
        )


def _emit(tc, t):
    nc = tc.nc
    fp, bf = F32, BF16

    with tc.tile_pool(name="cst", bufs=1) as cst:
        # ---- residents
        xb0 = cst.tile([P0, N], bf, tag="xb0")
        nc.sync.dma_start(out=xb0, in_=t["xb"][0:P0, :])
        xb1 = cst.tile([P1, N], bf, tag="xb1")
        nc.sync.dma_start(out=xb1, in_=t["xb"][P0:DIM, :])

        def wload(name, rows0, rows1):
            w0 = cst.tile([P0, DIM], bf, tag=name + "0")
            nc.sync.dma_start(out=w0, in_=t[name][0:P0, :])
            w1 = cst.tile([P1, DIM], bf, tag=name + "1")
            nc.sync.dma_start(out=w1, in_=t[name][P0:DIM, :])
            return w0, w1

        wq0, wq1 = wload("wq", P0, P1)
        wk0, wk1 = wload("wk", P0, P1)
        wv0, wv1 = wload("wv", P0, P1)
        wpoA = cst.tile([96, DIM], bf, tag="wpoA")
        nc.sync.dma_start(out=wpoA, in_=t["wpo"][0:96, :])
        wpoB = cst.tile([96, DIM], bf, tag="wpoB")
        nc.sync.dma_start(out=wpoB, in_=t["wpo"][96:DIM, :])

        def dload(name, p):
            d = cst.tile([p, 9, p], bf, tag=name)
            nc.sync.dma_start(out=d, in_=t[name])
            return d

        dq0 = dload("dq0", P0)
        dk0 = dload("dk0", P0)
        dqk1 = dload("dqk1", P0)
        dv0 = dload("dv0", 96)
        dv1 = dload("dv1", 96)
        idn = cst.tile([P0, P0], bf, tag="idn")
        nc.sync.dma_start(out=idn, in_=t["ident"])
        tsc0 = cst.tile([P0, 1], fp, tag="tsc0")
        nc.sync.dma_start(out=tsc0, in_=t["tsc"][0:P0, :])
        tsc1 = cst.tile([P1, 1], fp, tag="tsc1")
        nc.sync.dma_start(out=tsc1, in_=t["tsc"][P0:DIM, :])

        qn_p0 = cst.tile([P0, NB], fp, tag="qn_p0")
        kn_p0 = cst.tile([P0, NB], fp, tag="kn_p0")
        qkn_p1 = cst.tile([P0, NB], fp, tag="qkn_p1")
        AT01 = cst.tile([96, 96], bf, tag="AT01")
        AT23 = cst.tile([96, 96], bf, tag="AT23")

        # ================= PASS A: q/k conv+dw+transpose, Gram, norms
        ring_q0, ring_k0, ring_m1 = [], [], []
        with tc.tile_pool(name="wa", bufs=CFG["wa"]) as wa, \
             tc.tile_pool(name="rng", bufs=CFG["rng"]) as rng, \
             tc.tile_pool(name="dsb", bufs=CFG["dsb"]) as dsb, \
             tc.tile_pool(name="scrp", bufs=CFG["scrp"]) as scrp, \
             tc.tile_pool(name="psc", bufs=CFG["psc"], space="PSUM") as psc, \
             tc.tile_pool(name="psd", bufs=1, space="PSUM") as psd, \
             tc.tile_pool(name="pst", bufs=CFG["pst"], space="PSUM") as pst, \
             tc.tile_pool(name="psg", bufs=1, space="PSUM") as psg:

            # head-pair Gram blocks GT[d, pair, c] (transposed logits):
            # pair 0 = k/q ch 0..95 (heads 0,1), pair 1 = ch 96..191 (heads 2,3)
            GT = psg.tile([96, 2, 96], fp, tag="GT")

            for i in range(NB + 1):
                if i < NB:
                    cols = slice(i * WB, (i + 1) * WB)
                    xa0 = wa.tile([P0, RB, W], bf, tag="xa0")
                    nc.sync.dma_start(out=xa0, in_=t["xa"][0:P0, cols])
                    xa1 = wa.tile([P1, RB, W], bf, tag="xa1")
                    nc.sync.dma_start(out=xa1, in_=t["xa"][P0:DIM, cols])

                    pq = psc.tile([P0, RB, W], fp, tag="conv")
                    nc.tensor.matmul(pq, wq0[:, 0:P0], xa0, start=True, stop=False)
                    nc.tensor.matmul(pq, wq1[:, 0:P0], xa1, start=False, stop=True)
                    q0 = rng.tile([P0, RB, W], bf, tag="q0")
                    ring_q0.append(q0)
                    nc.vector.tensor_copy(out=q0, in_=pq)

                    pk = psc.tile([P0, RB, W], fp, tag="conv")
                    nc.tensor.matmul(pk, wk0[:, 0:P0], xb0[:, cols], start=True, stop=False)
                    nc.tensor.matmul(pk, wk1[:, 0:P0], xb1[:, cols], start=False, stop=True)
                    k0 = rng.tile([P0, RB, W], bf, tag="k0")
                    ring_k0.append(k0)
                    nc.vector.tensor_copy(out=k0, in_=pk)

                    # merged chunk1: partitions 0:64 = q ch 128..191, 64:128 = k ch 128..191
                    pm = psc.tile([P0, RB, W], fp, tag="conv")
                    nc.tensor.matmul(pm[0:P1], wq0[:, P0:DIM], xa0, start=True, stop=False)
                    nc.tensor.matmul(pm[0:P1], wq1[:, P0:DIM], xa1, start=False, stop=True)
                    nc.tensor.matmul(pm[P1:P0], wk0[:, P0:DIM], xb0[:, cols], start=True, stop=False)
                    nc.tensor.matmul(pm[P1:P0], wk1[:, P0:DIM], xb1[:, cols], start=False, stop=True)
                    m1 = rng.tile([P0, RB, W], bf, tag="m1")
                    ring_m1.append(m1)
                    nc.vector.tensor_copy(out=m1, in_=pm)

                if i >= 1:
                    j = i - 1
                    pdq = psd.tile([P0, RB, W], fp, tag="dwq", bufs=CFG["dwq2"])
                    _emit_dw(nc, pdq, ring_q0, dq0, j)
                    pdk = psd.tile([P0, RB, W], fp, tag="dwk", bufs=CFG["dwk2"])
                    _emit_dw(nc, pdk, ring_k0, dk0, j)
                    pdm = psd.tile([P0, RB, W], fp, tag="dwm", bufs=CFG["dwm2"])
                    _emit_dw(nc, pdm, ring_m1, dqk1, j)

                    qdw = dsb.tile([P0, RB, W], bf, tag="qdw")
                    nc.scalar.copy(out=qdw, in_=pdq)
                    kdw = dsb.tile([P0, RB, W], bf, tag="kdw")
                    nc.scalar.copy(out=kdw, in_=pdk)
                    mdw = dsb.tile([P0, RB, W], bf, tag="mdw")
                    nc.scalar.copy(out=mdw, in_=pdm)

                    scr = scrp.tile([P0, RB, W], fp, tag="scr")
                    nc.scalar.activation(out=scr, in_=pdq, func=AF.Square,
                                         accum_out=qn_p0[:, j:j + 1])
                    scr = scrp.tile([P0, RB, W], fp, tag="scr")
                    nc.scalar.activation(out=scr, in_=pdk, func=AF.Square,
                                         accum_out=kn_p0[:, j:j + 1])
                    scr = scrp.tile([P0, RB, W], fp, tag="scr")
                    nc.scalar.activation(out=scr, in_=pdm, func=AF.Square,
                                         accum_out=qkn_p1[:, j:j + 1])

                    qT = dsb.tile([P0, RB, DIM], bf, tag="qT")
                    kT = dsb.tile([P0, RB, DIM], bf, tag="kT")
                    for y in range(RB):
                        tp = pst.tile([P0, P0], bf, tag="tp")
                        nc.tensor.transpose(tp, qdw[:, y, :], idn)
                        nc.vector.tensor_copy(out=qT[:, y, 0:P0], in_=tp)
                        tp = pst.tile([P0, P0], bf, tag="tp")
                        nc.tensor.transpose(tp, kdw[:, y, :], idn)
                        nc.vector.tensor_copy(out=kT[:, y, 0:P0], in_=tp)
                        tp = pst.tile([P0, P0], bf, tag="tp")
                        nc.tensor.transpose(tp, mdw[:, y, :], idn)
                        nc.vector.tensor_copy(out=qT[:, y, P0:DIM], in_=tp[:, 0:P1])
                        nc.vector.tensor_copy(out=kT[:, y, P0:DIM], in_=tp[:, P1:P0])

                        for p in range(2):
                            ps_ = slice(96 * p, 96 * p + 96)
                            nc.tensor.matmul(GT[:, p, :], kT[:, y, ps_], qT[:, y, ps_],
                                             start=(j == 0 and y == 0 and p == 0),
                                             stop=(j == NB - 1 and y == RB - 1 and p == 1),
                                             skip_group_check=True)

            # ============ interlude: norms, logits scaling, softmax, A^T
            def rnorm(src, nm):
                d = cst.tile(list(src.shape[:1]) + [1], fp, tag=nm, name=nm)
                nc.scalar.activation(out=d, in_=src, func=AF.Sqrt)
                nc.vector.tensor_scalar_max(out=d, in0=d, scalar1=EPS)
                nc.vector.reciprocal(out=d, in_=d)
                return d

            qn2_0 = cst.tile([P0, 1], fp, tag="qn2_0")
            nc.vector.tensor_reduce(out=qn2_0, in_=qn_p0, axis=AX.X, op=OP.add)
            kn2_0 = cst.tile([P0, 1], fp, tag="kn2_0")
            nc.vector.tensor_reduce(out=kn2_0, in_=kn_p0, axis=AX.X, op=OP.add)
            mn2 = cst.tile([P0, 1], fp, tag="mn2")
            nc.vector.tensor_reduce(out=mn2, in_=qkn_p1, axis=AX.X, op=OP.add)

            rq0 = rnorm(qn2_0, "rq0")                       # [128,1] 1/||q|| ch 0..127
            rk0 = rnorm(kn2_0, "rk0")
            rm = rnorm(mn2, "rm")                          # [0:64]=q ch128.., [64:128]=k ch128..
            nc.vector.tensor_mul(out=rq0, in0=rq0, in1=tsc0)   # fold temperature
            nc.vector.tensor_mul(out=rm[0:P1], in0=rm[0:P1], in1=tsc1)

            # per-pair scale vectors at partition base 0 (cross-partition -> SWDGE DMA)
            # q channels: 0..127 in rq0, 128..191 in rm[0:64]
            # k channels: 0..127 in rk0, 128..191 in rm[64:128]
            rq_hi = cst.tile([96, 1], fp, tag="rq_hi")
            rk_hi = cst.tile([96, 1], fp, tag="rk_hi")
            nc.gpsimd.dma_start(out=rq_hi[0:32], in_=rq0[96:P0])
            nc.gpsimd.dma_start(out=rq_hi[32:96], in_=rm[0:P1])
            nc.gpsimd.dma_start(out=rk_hi[0:32], in_=rk0[96:P0])
            nc.gpsimd.dma_start(out=rk_hi[32:96], in_=rm[P1:P0])

            for p, (rkx, rqx, ATdst) in enumerate(
                    ((rk0[0:96], rq0[0:96], AT01), (rk_hi, rq_hi, AT23))):
                GTsb = cst.tile([96, 96], bf, tag=f"GTsb{p}", name=f"GTsb{p}")
                nc.vector.tensor_scalar_mul(out=GTsb, in0=GT[:, p, :], scalar1=rkx)
                Gp = pst.tile([P0, P0], bf, tag="tp")
                nc.tensor.transpose(Gp[0:96, 0:96], GTsb, idn[0:96, 0:96])
                Gsb = cst.tile([96, 96], fp, tag=f"Gsb{p}", name=f"Gsb{p}")
                nc.vector.tensor_scalar_mul(out=Gsb, in0=Gp[0:96, 0:96], scalar1=rqx)
                # head 1 of the pair sits at partition base 48 (not 32-aligned):
                # extract its [48,48] block to base 0 via SWDGE
                G1 = cst.tile([HD, HD], fp, tag=f"G1{p}", name=f"G1{p}")
                nc.gpsimd.dma_start(out=G1, in_=Gsb[HD:96, HD:96])

                nc.vector.memset(ATdst, 0.0)
                for hh, blk in ((0, Gsb[0:HD, 0:HD]), (1, G1)):
                    mneg = cst.tile([HD, 1], fp, tag=f"mneg{p}{hh}", name=f"mneg{p}{hh}")
                    nc.vector.tensor_reduce(out=mneg, in_=blk, axis=AX.X,
                                            op=OP.max, negate=True)
                    E = cst.tile([HD, HD], fp, tag=f"E{p}{hh}", name=f"E{p}{hh}")
                    nc.scalar.activation(out=E, in_=blk, func=AF.Exp,
                                         bias=mneg, scale=1.0)
                    sm = cst.tile([HD, 1], fp, tag=f"sm{p}{hh}", name=f"sm{p}{hh}")
                    nc.vector.tensor_reduce(out=sm, in_=E, axis=AX.X, op=OP.add)
                    rs = cst.tile([HD, 1], fp, tag=f"rs{p}{hh}", name=f"rs{p}{hh}")
                    nc.vector.reciprocal(out=rs, in_=sm)
                    Ah = cst.tile([HD, HD], bf, tag=f"Ah{p}{hh}", name=f"Ah{p}{hh}")
                    nc.vector.tensor_scalar_mul(out=Ah, in0=E, scalar1=rs)
                    ATp = pst.tile([P0, P0], bf, tag="tp")
                    nc.tensor.transpose(ATp[0:HD, 0:HD], Ah, idn[0:HD, 0:HD])
                    if hh == 0:
                        nc.scalar.copy(out=ATdst[0:HD, 0:HD], in_=ATp[0:HD, 0:HD])
                    else:
                        tmp = cst.tile([HD, HD], bf, tag=f"ATt{p}", name=f"ATt{p}")
                        nc.scalar.copy(out=tmp, in_=ATp[0:HD, 0:HD])
                        nc.gpsimd.dma_start(out=ATdst[HD:96, HD:96], in_=tmp)

        # ================= PASS B: v conv+dw, attn@v, po, out
        ring_v0, ring_v1 = [], []
        with tc.tile_pool(name="rngB", bufs=CFG["rngB"]) as rngB, \
             tc.tile_pool(name="dsbB", bufs=CFG["dsbB"]) as dsbB, \
             tc.tile_pool(name="osb", bufs=CFG["osb"]) as osb, \
             tc.tile_pool(name="pcv", bufs=CFG["pcv"], space="PSUM") as pcv, \
             tc.tile_pool(name="pdv", bufs=1, space="PSUM") as pdv, \
             tc.tile_pool(name="pav", bufs=1, space="PSUM") as pav, \
             tc.tile_pool(name="ppo", bufs=1, space="PSUM") as ppo:
            for i in range(NB + 1):
                if i < NB:
                    cols = slice(i * WB, (i + 1) * WB)
                    pv0 = pcv.tile([96, RB, W], fp, tag="cv")
                    nc.tensor.matmul(pv0, wv0[:, 0:96], xb0[:, cols], start=True, stop=False)
                    nc.tensor.matmul(pv0, wv1[:, 0:96], xb1[:, cols], start=False, stop=True)
                    v0 = rngB.tile([96, RB, W], bf, tag="v0")
                    ring_v0.append(v0)
                    nc.vector.tensor_copy(out=v0, in_=pv0)
                    pv1 = pcv.tile([96, RB, W], fp, tag="cv")
                    nc.tensor.matmul(pv1, wv0[:, 96:DIM], xb0[:, cols], start=True, stop=False)
                    nc.tensor.matmul(pv1, wv1[:, 96:DIM], xb1[:, cols], start=False, stop=True)
                    v1 = rngB.tile([96, RB, W], bf, tag="v1")
                    ring_v1.append(v1)
                    nc.vector.tensor_copy(out=v1, in_=pv1)
                if i >= 1:
                    j = i - 1
                    colj = slice(j * WB, (j + 1) * WB)
                    pd0 = pdv.tile([96, RB, W], fp, tag="dv0")
                    _emit_dw(nc, pd0, ring_v0, dv0, j)
                    vd0 = dsbB.tile([96, RB, W], bf, tag="vd0")
                    nc.vector.tensor_copy(out=vd0, in_=pd0)
                    pd1 = pdv.tile([96, RB, W], fp, tag="dv1")
                    _emit_dw(nc, pd1, ring_v1, dv1, j)
                    vd1 = dsbB.tile([96, RB, W], bf, tag="vd1")
                    nc.vector.tensor_copy(out=vd1, in_=pd1)

                    pa1 = pav.tile([96, RB, W], fp, tag="av01")
                    nc.tensor.matmul(pa1, AT01, vd0, start=True, stop=True)
                    pa2 = pav.tile([96, RB, W], fp, tag="av23")
                    nc.tensor.matmul(pa2, AT23, vd1, start=True, stop=True)
                    a1 = dsbB.tile([96, RB, W], bf, tag="a1")
                    nc.scalar.copy(out=a1, in_=pa1)
                    a2 = dsbB.tile([96, RB, W], bf, tag="a2")
                    nc.scalar.copy(out=a2, in_=pa2)

                    pp0 = ppo.tile([P0, RB, W], fp, tag="po0")
                    nc.tensor.matmul(pp0, wpoA[:, 0:P0], a1, start=True, stop=False)
                    nc.tensor.matmul(pp0, wpoB[:, 0:P0], a2, start=False, stop=True)
                    pp1 = ppo.tile([P1, RB, W], fp, tag="po1")
                    nc.tensor.matmul(pp1, wpoA[:, P0:DIM], a1, start=True, stop=False)
                    nc.tensor.matmul(pp1, wpoB[:, P0:DIM], a2, start=False, stop=True)
                    o0 = osb.tile([P0, RB, W], fp, tag="o0")
                    nc.scalar.copy(out=o0, in_=pp0)
                    o1 = osb.tile([P1, RB, W], fp, tag="o1")
                    nc.vector.tensor_copy(out=o1, in_=pp1)
                    nc.sync.dma_start(out=t["out"][0:P0, colj], in_=o0)
                    nc.sync.dma_start(out=t["out"][P0:DIM, colj], in_=o1)


def _build():
    nc = bacc.Bacc("TRN2", target_bir_lowering=False, debug=False, num_devices=8)
    t = {}

    def din(name, shape, dt):
        t[name] = nc.dram_tensor(name, shape, dt, kind="ExternalInput").ap()

    din("xa", [DIM, N], BF16)
    din("xb", [DIM, N], BF16)
    din("wq", [DIM, DIM], BF16)
    din("wk", [DIM, DIM], BF16)
    din("wv", [DIM, DIM], BF16)
    din("wpo", [DIM, DIM], BF16)
    din("dq0", [P0, 9, P0], BF16)
    din("dk0", [P0, 9, P0], BF16)
    din("dqk1", [P0, 9, P0], BF16)
    din("dv0", [96, 9, 96], BF16)
    din("dv1", [96, 9, 96], BF16)
    din("tsc", [DIM, 1], F32)
    din("ident", [P0, P0], BF16)
    t["out"] = nc.dram_tensor("out", [DIM, N], F32, kind="ExternalOutput").ap()

    with tile.TileContext(nc) as tc:
        _emit(tc, t)
    nc.compile()
    return nc


def _diag_pack(w, p):
    """w [p, 9] -> [p, 9, p] with m[c, t, c] = w[c, t]."""
    m = np.zeros((p, 9, p), np.float32)
    i = np.arange(p)
    m[i, :, i] = w
    return m.astype(BF)


def _prep_core(inp, c):
    b, br = divmod(c, 2)
    x = np.asarray(inp["x"], np.float32)
    cx = np.asarray(inp["ctx"], np.float32)
    if br == 0:  # ctx branch -> ctx_out[b]: q from x, k/v from ctx
        A, Bm = x[b], cx[b]
        wqkv_a, dw_a = inp["x_qkv_w"], inp["x_dw_w"]
        wqkv_b, dw_b = inp["ctx_qkv_w"], inp["ctx_dw_w"]
        po = inp["ctx_po_w"]
    else:        # x branch -> x_out[b]: q from ctx, k/v from x
        A, Bm = cx[b], x[b]
        wqkv_a, dw_a = inp["ctx_qkv_w"], inp["ctx_dw_w"]
        wqkv_b, dw_b = inp["x_qkv_w"], inp["x_dw_w"]
        po = inp["x_po_w"]
    wqkv_a = np.asarray(wqkv_a, np.float32)[:, :, 0, 0]
    wqkv_b = np.asarray(wqkv_b, np.float32)[:, :, 0, 0]
    dw_a = np.asarray(dw_a, np.float32)[:, 0].reshape(3 * DIM, 9)
    dw_b = np.asarray(dw_b, np.float32)[:, 0].reshape(3 * DIM, 9)
    po = np.asarray(po, np.float32)[:, :, 0, 0]
    temp = np.asarray(inp["temperature"], np.float32).reshape(NH)

    dq = dw_a[0:DIM]
    dk = dw_b[DIM:2 * DIM]
    dv = dw_b[2 * DIM:3 * DIM]
    m1 = np.zeros((P0, 9, P0), np.float32)
    i64 = np.arange(P1)
    m1[i64, :, i64] = dq[P0:DIM]
    m1[i64 + P1, :, i64 + P1] = dk[P0:DIM]

    return {
        "xa": np.ascontiguousarray(A.reshape(DIM, N)).astype(BF),
        "xb": np.ascontiguousarray(Bm.reshape(DIM, N)).astype(BF),
        "wq": np.ascontiguousarray(wqkv_a[0:DIM].T).astype(BF),
        "wk": np.ascontiguousarray(wqkv_b[DIM:2 * DIM].T).astype(BF),
        "wv": np.ascontiguousarray(wqkv_b[2 * DIM:3 * DIM].T).astype(BF),
        "wpo": np.ascontiguousarray(po.T).astype(BF),
        "dq0": _diag_pack(dq[0:P0], P0),
        "dk0": _diag_pack(dk[0:P0], P0),
        "dqk1": m1.astype(BF),
        "dv0": _diag_pack(dv[0:96], 96),
        "dv1": _diag_pack(dv[96:DIM], 96),
        "tsc": np.repeat(temp, HD)[:, None].astype(np.float32),
        "ident": np.eye(P0, dtype=np.float32).astype(BF),
    }


_BUILT = None
LAST_RESULTS = None


def kernel(**inputs):
    global _BUILT, LAST_RESULTS
    if _BUILT is None:
        _BUILT = _build()
    in_maps = [_prep_core(inputs, c) for c in range(8)]
    res = run_bass_kernel_spmd(
        _BUILT, in_maps, list(range(8)),
        trace=bool(int(os.environ.get("KTRACE", "0"))),
    )
    LAST_RESULTS = res
    outs = [np.asarray(r["out"], np.float32) for r in res.results]
    x_out = np.stack([outs[2 * b + 1].reshape(DIM, H, W) for b in range(4)])
    ctx_out = np.stack([outs[2 * b].reshape(DIM, H, W) for b in range(4)])
    return (x_out, ctx_out)


# revision 32
# speedup vs baseline: 240.9836x; 1.1331x over previous
"""Trainium2 Bass kernel for nn_CrossAttention (XCA-style cross channel-attention).

Sharding: 8 cores = (batch b, branch). branch 0 computes ctx_out[b] (q from x,
k/v from ctx); branch 1 computes x_out[b] (q from ctx, k/v from x). Each core:
  conv1x1 qkv (PE) -> depthwise 3x3 (PE, diagonal-weight matmuls with shifted
  access patterns + PSUM accumulation) -> l2norm (ACT square-accum + small ops)
  -> per-head channel Gram (PE, transposed layout) -> softmax (DVE/ACT) ->
  attn@v (PE) -> conv1x1 po (PE) -> DMA out.
Compute in bf16 with fp32 PSUM accumulation; norms/softmax in fp32.
"""

import os
import numpy as np
import ml_dtypes

import concourse.bass as bass
import concourse.tile as tile
from concourse import bacc, mybir
from concourse.bass_utils import run_bass_kernel_spmd

BF = ml_dtypes.bfloat16
F32 = mybir.dt.float32
BF16 = mybir.dt.bfloat16

DIM = 192
H = W = 128
N = H * W
NH, HD = 4, 48
P0, P1 = 128, 64
RB = 4              # image rows per block
WB = RB * W         # 512 free elems per block
NB = H // RB        # 32 blocks
EPS = 1e-12

CFG = {"wa": 3, "rng": 4, "dsb": 2, "scrp": 2, "scrb": 2, "psc": 2, "pst": 2,
       "vtA": 1, "vtB": 1,
       "rngB": 4, "dsbB": 3, "osb": 2, "pcv": 2,
       "dwq2": 1, "dwk2": 1, "dwm2": 1}

AF = mybir.ActivationFunctionType
AX = mybir.AxisListType
OP = mybir.AluOpType


def _dw_segments(j):
    """Matmul segments for 3x3 depthwise conv of block j (rows j*RB..j*RB+RB).

    Returns list of (tap, src_block, y0, y1, r0, r1, sx0, sx1, ox0, ox1):
    out[:, y0:y1, ox0:ox1] += diag(w_tap) @ src[src_block][:, r0:r1, sx0:sx1]
    Center tap (full coverage) is first.
    """
    segs = []
    for ky in range(3):
        for kx in range(3):
            dy, dx = ky - 1, kx - 1
            tt = ky * 3 + kx
            ox0, ox1 = max(0, -dx), W - max(0, dx)
            sx0, sx1 = ox0 + dx, ox1 + dx
            ylo = max(j * RB, max(0, -dy))
            yhi = min(j * RB + RB, H - max(0, dy))
            Y = ylo
            while Y < yhi:
                jj = (Y + dy) // RB
                yend = min(yhi, (jj + 1) * RB - dy)
                r0 = (Y + dy) % RB
                if dx == 0:
                    # full-width rows are contiguous: one matmul per segment
                    segs.append((tt, jj, Y - j * RB, yend - j * RB,
                                 r0, r0 + (yend - Y), sx0, sx1, ox0, ox1))
                else:
                    # shifted columns: per-row matmuls (sim/HW want flat free APs)
                    for yy in range(Y, yend):
                        rr = r0 + (yy - Y)
                        segs.append((tt, jj, yy - j * RB, yy - j * RB + 1,
                                     rr, rr + 1, sx0, sx1, ox0, ox1))
                Y = yend
    segs.sort(key=lambda s: 0 if s[0] == 4 else 1)
    return segs


def _emit_dw(nc, pd, ring, dg, j, skip_taps=()):
    segs = [s for s in _dw_segments(j) if s[0] not in skip_taps]
    for idx, (tt, jj, y0, y1, r0, r1, sx0, sx1, ox0, ox1) in enumerate(segs):
        nc.tensor.matmul(
            pd[:, y0:y1, ox0:ox1],
            dg[:, tt, :],
            ring[jj][:, r0:r1, sx0:sx1],
            start=(idx == 0),
            stop=(idx == len(segs) - 1),
        )


def _emit_dw_vtaps_dve(nc, tmp_a, tmp_b, ring, wcol, j):
    """Vertical taps as DVE per-partition scaled copies (bf16 4x mode)."""
    nc.vector.tensor_scalar_mul(out=tmp_a[:, 0, :], in0=ring[j - 1][:, RB - 1, :],
                                scalar1=wcol[:, 1:2])
    nc.vector.tensor_scalar_mul(out=tmp_a[:, 1:RB, :], in0=ring[j][:, 0:RB - 1, :],
                                scalar1=wcol[:, 1:2])
    nc.vector.tensor_scalar_mul(out=tmp_b[:, 0:RB - 1, :], in0=ring[j][:, 1:RB, :],
                                scalar1=wcol[:, 7:8])
    nc.vector.tensor_scalar_mul(out=tmp_b[:, RB - 1, :], in0=ring[j + 1][:, 0, :],
                                scalar1=wcol[:, 7:8])


def _emit_dw_vtaps(nc, tmp_a, tmp_b, ring, wcol, j):
    """Vertical taps (dy=-1 tap 1, dy=+1 tap 7) as ACT per-partition scaled
    copies into tmp_a/tmp_b. Interior blocks only (1 <= j <= NB-2)."""
    # tap 1: out row y <- src row y-1
    nc.scalar.activation(out=tmp_a[:, 0, :], in_=ring[j - 1][:, RB - 1, :],
                         func=AF.Copy, scale=wcol[:, 1:2])
    nc.scalar.activation(out=tmp_a[:, 1:RB, :], in_=ring[j][:, 0:RB - 1, :],
                         func=AF.Copy, scale=wcol[:, 1:2])
    # tap 7: out row y <- src row y+1
    nc.scalar.activation(out=tmp_b[:, 0:RB - 1, :], in_=ring[j][:, 1:RB, :],
                         func=AF.Copy, scale=wcol[:, 7:8])
    nc.scalar.activation(out=tmp_b[:, RB - 1, :], in_=ring[j + 1][:, 0, :],
                         func=AF.Copy, scale=wcol[:, 7:8])


def _emit(tc, t):
    nc = tc.nc
    fp, bf = F32, BF16

    with tc.tile_pool(name="cst", bufs=1) as cst:
        # ---- residents
        xb0 = cst.tile([P0, N], bf, tag="xb0")
        nc.sync.dma_start(out=xb0, in_=t["xb"][0:P0, :])
        xb1 = cst.tile([P1, N], bf, tag="xb1")
        nc.sync.dma_start(out=xb1, in_=t["xb"][P0:DIM, :])

        def wload(name, rows0, rows1):
            w0 = cst.tile([P0, DIM], bf, tag=name + "0")
            nc.sync.dma_start(out=w0, in_=t[name][0:P0, :])
            w1 = cst.tile([P1, DIM], bf, tag=name + "1")
            nc.sync.dma_start(out=w1, in_=t[name][P0:DIM, :])
            return w0, w1

        wq0, wq1 = wload("wq", P0, P1)
        wk0, wk1 = wload("wk", P0, P1)
        wv0, wv1 = wload("wv", P0, P1)
        wpoA = cst.tile([96, DIM], bf, tag="wpoA")
        nc.sync.dma_start(out=wpoA, in_=t["wpo"][0:96, :])
        wpoB = cst.tile([96, DIM], bf, tag="wpoB")
        nc.sync.dma_start(out=wpoB, in_=t["wpo"][96:DIM, :])

        def dload(name, p):
            d = cst.tile([p, 9, p], bf, tag=name)
            nc.sync.dma_start(out=d, in_=t[name])
            return d

        dq0 = dload("dq0", P0)
        dk0 = dload("dk0", P0)
        dqk1 = dload("dqk1", P0)
        dv0 = dload("dv0", 96)
        dv1 = dload("dv1", 96)
        idn = cst.tile([P0, P0], bf, tag="idn")
        nc.sync.dma_start(out=idn, in_=t["ident"])
        tsc0 = cst.tile([P0, 1], fp, tag="tsc0")
        nc.sync.dma_start(out=tsc0, in_=t["tsc"][0:P0, :])
        tsc1 = cst.tile([P1, 1], fp, tag="tsc1")
        nc.sync.dma_start(out=tsc1, in_=t["tsc"][P0:DIM, :])

        def wcload(name, p):
            wtile = cst.tile([p, 9], fp, tag=name, name=name)
            nc.sync.dma_start(out=wtile, in_=t[name])
            return wtile

        wc_q0 = wcload("wc_q0", P0)
        wc_k0 = wcload("wc_k0", P0)
        wc_m1 = wcload("wc_m1", P0)
        wc_v0 = wcload("wc_v0", 96)
        wc_v1 = wcload("wc_v1", 96)

        qn_p0 = cst.tile([P0, NB], fp, tag="qn_p0")
        kn_p0 = cst.tile([P0, NB], fp, tag="kn_p0")
        qkn_p1 = cst.tile([P0, NB], fp, tag="qkn_p1")
        AT01 = cst.tile([96, 96], bf, tag="AT01")
        AT23 = cst.tile([96, 96], bf, tag="AT23")

        # ================= PASS A: q/k conv+dw+transpose, Gram, norms
        ring_q0, ring_k0, ring_m1 = [], [], []
        with tc.tile_pool(name="wa", bufs=CFG["wa"]) as wa, \
             tc.tile_pool(name="rng", bufs=CFG["rng"]) as rng, \
             tc.tile_pool(name="dsb", bufs=CFG["dsb"]) as dsb, \
             tc.tile_pool(name="scrp", bufs=CFG["scrp"]) as scrp, \
             tc.tile_pool(name="scrb", bufs=CFG["scrb"]) as scrb, \
             tc.tile_pool(name="psc", bufs=CFG["psc"], space="PSUM") as psc, \
             tc.tile_pool(name="psd", bufs=1, space="PSUM") as psd, \
             tc.tile_pool(name="pst", bufs=CFG["pst"], space="PSUM") as pst, \
             tc.tile_pool(name="psg", bufs=1, space="PSUM") as psg:

            # head-pair Gram blocks GT[d, pair, c] (transposed logits):
            # pair 0 = k/q ch 0..95 (heads 0,1), pair 1 = ch 96..191 (heads 2,3)
            GT = psg.tile([96, 2, 96], fp, tag="GT")

            for i in range(NB + 1):
                if i < NB:
                    cols = slice(i * WB, (i + 1) * WB)
                    xa0 = wa.tile([P0, RB, W], bf, tag="xa0")
                    nc.sync.dma_start(out=xa0, in_=t["xa"][0:P0, cols])
                    xa1 = wa.tile([P1, RB, W], bf, tag="xa1")
                    nc.sync.dma_start(out=xa1, in_=t["xa"][P0:DIM, cols])

                    pq = psc.tile([P0, RB, W], fp, tag="conv")
                    nc.tensor.matmul(pq, wq0[:, 0:P0], xa0, start=True, stop=False)
                    nc.tensor.matmul(pq, wq1[:, 0:P0], xa1, start=False, stop=True)
                    q0 = rng.tile([P0, RB, W], bf, tag="q0")
                    ring_q0.append(q0)
                    nc.scalar.copy(out=q0, in_=pq)

                    pk = psc.tile([P0, RB, W], fp, tag="conv")
                    nc.tensor.matmul(pk, wk0[:, 0:P0], xb0[:, cols], start=True, stop=False)
                    nc.tensor.matmul(pk, wk1[:, 0:P0], xb1[:, cols], start=False, stop=True)
                    k0 = rng.tile([P0, RB, W], bf, tag="k0")
                    ring_k0.append(k0)
                    nc.scalar.copy(out=k0, in_=pk)

                    # merged chunk1: partitions 0:64 = q ch 128..191, 64:128 = k ch 128..191
                    pm = psc.tile([P0, RB, W], fp, tag="conv")
                    nc.tensor.matmul(pm[0:P1], wq0[:, P0:DIM], xa0, start=True, stop=False)
                    nc.tensor.matmul(pm[0:P1], wq1[:, P0:DIM], xa1, start=False, stop=True)
                    nc.tensor.matmul(pm[P1:P0], wk0[:, P0:DIM], xb0[:, cols], start=True, stop=False)
                    nc.tensor.matmul(pm[P1:P0], wk1[:, P0:DIM], xb1[:, cols], start=False, stop=True)
                    m1 = rng.tile([P0, RB, W], bf, tag="m1")
                    ring_m1.append(m1)
                    nc.vector.tensor_copy(out=m1, in_=pm)

                if i >= 1:
                    j = i - 1
                    interior = CFG["vtA"] and 1 <= j <= NB - 2
                    skip = (1, 7) if interior else ()
                    pdq = psd.tile([P0, RB, W], fp, tag="dwq", bufs=CFG["dwq2"])
                    _emit_dw(nc, pdq, ring_q0, dq0, j, skip)
                    pdk = psd.tile([P0, RB, W], fp, tag="dwk", bufs=CFG["dwk2"])
                    _emit_dw(nc, pdk, ring_k0, dk0, j, skip)
                    pdm = psd.tile([P0, RB, W], fp, tag="dwm", bufs=CFG["dwm2"])
                    _emit_dw(nc, pdm, ring_m1, dqk1, j, skip)

                    qdw = dsb.tile([P0, RB, W], bf, tag="qdw")
                    nc.scalar.copy(out=qdw, in_=pdq)
                    kdw = dsb.tile([P0, RB, W], bf, tag="kdw")
                    nc.scalar.copy(out=kdw, in_=pdk)
                    mdw = dsb.tile([P0, RB, W], bf, tag="mdw")
                    nc.scalar.copy(out=mdw, in_=pdm)

                    if interior:
                        for (dst, ring_, wcol) in ((qdw, ring_q0, wc_q0),
                                                   (kdw, ring_k0, wc_k0),
                                                   (mdw, ring_m1, wc_m1)):
                            ta = scrb.tile([P0, RB, W], bf, tag="ta")
                            tb = scrb.tile([P0, RB, W], bf, tag="tb")
                            _emit_dw_vtaps_dve(nc, ta, tb, ring_, wcol, j)
                            acc = scrb.tile([P0, RB, W], bf, tag="acc")
                            nc.vector.tensor_add(out=acc, in0=ta, in1=tb)
                            nc.vector.tensor_add(out=dst, in0=dst, in1=acc)

                    scr = scrp.tile([P0, RB, W], fp, tag="scr")
                    nc.scalar.activation(out=scr, in_=qdw, func=AF.Square,
                                         accum_out=qn_p0[:, j:j + 1])
                    scr = scrp.tile([P0, RB, W], fp, tag="scr")
                    nc.scalar.activation(out=scr, in_=kdw, func=AF.Square,
                                         accum_out=kn_p0[:, j:j + 1])
                    scr = scrp.tile([P0, RB, W], fp, tag="scr")
                    nc.scalar.activation(out=scr, in_=mdw, func=AF.Square,
                                         accum_out=qkn_p1[:, j:j + 1])

                    qT = dsb.tile([P0, RB, DIM], bf, tag="qT")
                    kT = dsb.tile([P0, RB, DIM], bf, tag="kT")
                    for y in range(RB):
                        tp = pst.tile([P0, P0], bf, tag="tp")
                        nc.tensor.transpose(tp, qdw[:, y, :], idn)
                        nc.vector.tensor_copy(out=qT[:, y, 0:P0], in_=tp)
                        tp = pst.tile([P0, P0], bf, tag="tp")
                        nc.tensor.transpose(tp, kdw[:, y, :], idn)
                        nc.vector.tensor_copy(out=kT[:, y, 0:P0], in_=tp)
                        tp = pst.tile([P0, P0], bf, tag="tp")
                        nc.tensor.transpose(tp, mdw[:, y, :], idn)
                        nc.vector.tensor_copy(out=qT[:, y, P0:DIM], in_=tp[:, 0:P1])
                        nc.vector.tensor_copy(out=kT[:, y, P0:DIM], in_=tp[:, P1:P0])

                        for p in range(2):
                            ps_ = slice(96 * p, 96 * p + 96)
                            nc.tensor.matmul(GT[:, p, :], kT[:, y, ps_], qT[:, y, ps_],
                                             start=(j == 0 and y == 0 and p == 0),
                                             stop=(j == NB - 1 and y == RB - 1 and p == 1),
                                             skip_group_check=True)

            # ============ interlude: norms, logits scaling, softmax, A^T
            def rnorm(src, nm):
                d = cst.tile(list(src.shape[:1]) + [1], fp, tag=nm, name=nm)
                nc.scalar.activation(out=d, in_=src, func=AF.Sqrt)
                nc.vector.tensor_scalar_max(out=d, in0=d, scalar1=EPS)
                nc.vector.reciprocal(out=d, in_=d)
                return d

            qn2_0 = cst.tile([P0, 1], fp, tag="qn2_0")
            nc.vector.tensor_reduce(out=qn2_0, in_=qn_p0, axis=AX.X, op=OP.add)
            kn2_0 = cst.tile([P0, 1], fp, tag="kn2_0")
            nc.vector.tensor_reduce(out=kn2_0, in_=kn_p0, axis=AX.X, op=OP.add)
            mn2 = cst.tile([P0, 1], fp, tag="mn2")
            nc.vector.tensor_reduce(out=mn2, in_=qkn_p1, axis=AX.X, op=OP.add)

            rq0 = rnorm(qn2_0, "rq0")                       # [128,1] 1/||q|| ch 0..127
            rk0 = rnorm(kn2_0, "rk0")
            rm = rnorm(mn2, "rm")                          # [0:64]=q ch128.., [64:128]=k ch128..
            nc.vector.tensor_mul(out=rq0, in0=rq0, in1=tsc0)   # fold temperature
            nc.vector.tensor_mul(out=rm[0:P1], in0=rm[0:P1], in1=tsc1)

            # per-pair scale vectors at partition base 0 (cross-partition -> SWDGE DMA)
            # q channels: 0..127 in rq0, 128..191 in rm[0:64]
            # k channels: 0..127 in rk0, 128..191 in rm[64:128]
            rq_hi = cst.tile([96, 1], fp, tag="rq_hi")
            rk_hi = cst.tile([96, 1], fp, tag="rk_hi")
            nc.gpsimd.dma_start(out=rq_hi[0:32], in_=rq0[96:P0])
            nc.gpsimd.dma_start(out=rq_hi[32:96], in_=rm[0:P1])
            nc.gpsimd.dma_start(out=rk_hi[0:32], in_=rk0[96:P0])
            nc.gpsimd.dma_start(out=rk_hi[32:96], in_=rm[P1:P0])

            for p, (rkx, rqx, ATdst) in enumerate(
                    ((rk0[0:96], rq0[0:96], AT01), (rk_hi, rq_hi, AT23))):
                GTsb = cst.tile([96, 96], bf, tag=f"GTsb{p}", name=f"GTsb{p}")
                nc.vector.tensor_scalar_mul(out=GTsb, in0=GT[:, p, :], scalar1=rkx)
                Gp = pst.tile([P0, P0], bf, tag="tp")
                nc.tensor.transpose(Gp[0:96, 0:96], GTsb, idn[0:96, 0:96])
                Gsb = cst.tile([96, 96], fp, tag=f"Gsb{p}", name=f"Gsb{p}")
                nc.vector.tensor_scalar_mul(out=Gsb, in0=Gp[0:96, 0:96], scalar1=rqx)
                # head 1 of the pair sits at partition base 48 (not 32-aligned):
                # extract its [48,48] block to base 0 via SWDGE
                G1 = cst.tile([HD, HD], fp, tag=f"G1{p}", name=f"G1{p}")
                nc.gpsimd.dma_start(out=G1, in_=Gsb[HD:96, HD:96])

                nc.vector.memset(ATdst, 0.0)
                for hh, blk in ((0, Gsb[0:HD, 0:HD]), (1, G1)):
                    mneg = cst.tile([HD, 1], fp, tag=f"mneg{p}{hh}", name=f"mneg{p}{hh}")
                    nc.vector.tensor_reduce(out=mneg, in_=blk, axis=AX.X,
                                            op=OP.max, negate=True)
                    E = cst.tile([HD, HD], fp, tag=f"E{p}{hh}", name=f"E{p}{hh}")
                    nc.scalar.activation(out=E, in_=blk, func=AF.Exp,
                                         bias=mneg, scale=1.0)
                    sm = cst.tile([HD, 1], fp, tag=f"sm{p}{hh}", name=f"sm{p}{hh}")
                    nc.vector.tensor_reduce(out=sm, in_=E, axis=AX.X, op=OP.add)
                    rs = cst.tile([HD, 1], fp, tag=f"rs{p}{hh}", name=f"rs{p}{hh}")
                    nc.vector.reciprocal(out=rs, in_=sm)
                    Ah = cst.tile([HD, HD], bf, tag=f"Ah{p}{hh}", name=f"Ah{p}{hh}")
                    nc.vector.tensor_scalar_mul(out=Ah, in0=E, scalar1=rs)
                    ATp = pst.tile([P0, P0], bf, tag="tp")
                    nc.tensor.transpose(ATp[0:HD, 0:HD], Ah, idn[0:HD, 0:HD])
                    if hh == 0:
                        nc.scalar.copy(out=ATdst[0:HD, 0:HD], in_=ATp[0:HD, 0:HD])
                    else:
                        tmp = cst.tile([HD, HD], bf, tag=f"ATt{p}", name=f"ATt{p}")
                        nc.scalar.copy(out=tmp, in_=ATp[0:HD, 0:HD])
                        nc.gpsimd.dma_start(out=ATdst[HD:96, HD:96], in_=tmp)

        # ================= PASS B: v conv+dw, attn@v, po, out
        ring_v0, ring_v1 = [], []
        with tc.tile_pool(name="rngB", bufs=CFG["rngB"]) as rngB, \
             tc.tile_pool(name="scrB", bufs=CFG["scrb"]) as scrB, \
             tc.tile_pool(name="dsbB", bufs=CFG["dsbB"]) as dsbB, \
             tc.tile_pool(name="osb", bufs=CFG["osb"]) as osb, \
             tc.tile_pool(name="pcv", bufs=CFG["pcv"], space="PSUM") as pcv, \
             tc.tile_pool(name="pdv", bufs=1, space="PSUM") as pdv, \
             tc.tile_pool(name="pav", bufs=1, space="PSUM") as pav, \
             tc.tile_pool(name="ppo", bufs=1, space="PSUM") as ppo:
            for i in range(NB + 1):
                if i < NB:
                    cols = slice(i * WB, (i + 1) * WB)
                    pv0 = pcv.tile([96, RB, W], fp, tag="cv")
                    nc.tensor.matmul(pv0, wv0[:, 0:96], xb0[:, cols], start=True, stop=False)
                    nc.tensor.matmul(pv0, wv1[:, 0:96], xb1[:, cols], start=False, stop=True)
                    v0 = rngB.tile([96, RB, W], bf, tag="v0")
                    ring_v0.append(v0)
                    nc.vector.tensor_copy(out=v0, in_=pv0)
                    pv1 = pcv.tile([96, RB, W], fp, tag="cv")
                    nc.tensor.matmul(pv1, wv0[:, 96:DIM], xb0[:, cols], start=True, stop=False)
                    nc.tensor.matmul(pv1, wv1[:, 96:DIM], xb1[:, cols], start=False, stop=True)
                    v1 = rngB.tile([96, RB, W], bf, tag="v1")
                    ring_v1.append(v1)
                    nc.vector.tensor_copy(out=v1, in_=pv1)
                if i >= 1:
                    j = i - 1
                    colj = slice(j * WB, (j + 1) * WB)
                    interior = CFG["vtB"] and 1 <= j <= NB - 2
                    skip = (1, 7) if interior else ()
                    pd0 = pdv.tile([96, RB, W], fp, tag="dv0")
                    _emit_dw(nc, pd0, ring_v0, dv0, j, skip)
                    vd0 = dsbB.tile([96, RB, W], bf, tag="vd0")
                    nc.vector.tensor_copy(out=vd0, in_=pd0)
                    pd1 = pdv.tile([96, RB, W], fp, tag="dv1")
                    _emit_dw(nc, pd1, ring_v1, dv1, j, skip)
                    vd1 = dsbB.tile([96, RB, W], bf, tag="vd1")
                    nc.vector.tensor_copy(out=vd1, in_=pd1)
                    if interior:
                        for (dst, ring_, wcol) in ((vd0, ring_v0, wc_v0),
                                                   (vd1, ring_v1, wc_v1)):
                            ta = scrB.tile([96, RB, W], bf, tag="ta")
                            tb = scrB.tile([96, RB, W], bf, tag="tb")
                            _emit_dw_vtaps(nc, ta, tb, ring_, wcol, j)
                            acc = scrB.tile([96, RB, W], bf, tag="acc")
                            nc.vector.tensor_add(out=acc, in0=ta, in1=tb)
                            nc.vector.tensor_add(out=dst, in0=dst, in1=acc)

                    pa1 = pav.tile([96, RB, W], fp, tag="av01")
                    nc.tensor.matmul(pa1, AT01, vd0, start=True, stop=True)
                    pa2 = pav.tile([96, RB, W], fp, tag="av23")
                    nc.tensor.matmul(pa2, AT23, vd1, start=True, stop=True)
                    a1 = dsbB.tile([96, RB, W], bf, tag="a1")
                    nc.scalar.copy(out=a1, in_=pa1)
                    a2 = dsbB.tile([96, RB, W], bf, tag="a2")
                    nc.scalar.copy(out=a2, in_=pa2)

                    pp0 = ppo.tile([P0, RB, W], fp, tag="po0")
                    nc.tensor.matmul(pp0, wpoA[:, 0:P0], a1, start=True, stop=False)
                    nc.tensor.matmul(pp0, wpoB[:, 0:P0], a2, start=False, stop=True)
                    pp1 = ppo.tile([P1, RB, W], fp, tag="po1")
                    nc.tensor.matmul(pp1, wpoA[:, P0:DIM], a1, start=True, stop=False)
                    nc.tensor.matmul(pp1, wpoB[:, P0:DIM], a2, start=False, stop=True)
                    o0 = osb.tile([P0, RB, W], fp, tag="o0")
                    nc.scalar.copy(out=o0, in_=pp0)
                    o1 = osb.tile([P1, RB, W], fp, tag="o1")
                    nc.vector.tensor_copy(out=o1, in_=pp1)
                    nc.sync.dma_start(out=t["out"][0:P0, colj], in_=o0)
                    nc.sync.dma_start(out=t["out"][P0:DIM, colj], in_=o1)


def _build():
    nc = bacc.Bacc("TRN2", target_bir_lowering=False, debug=False, num_devices=8)
    t = {}

    def din(name, shape, dt):
        t[name] = nc.dram_tensor(name, shape, dt, kind="ExternalInput").ap()

    din("xa", [DIM, N], BF16)
    din("xb", [DIM, N], BF16)
    din("wq", [DIM, DIM], BF16)
    din("wk", [DIM, DIM], BF16)
    din("wv", [DIM, DIM], BF16)
    din("wpo", [DIM, DIM], BF16)
    din("dq0", [P0, 9, P0], BF16)
    din("dk0", [P0, 9, P0], BF16)
    din("dqk1", [P0, 9, P0], BF16)
    din("dv0", [96, 9, 96], BF16)
    din("dv1", [96, 9, 96], BF16)
    din("tsc", [DIM, 1], F32)
    din("wc_q0", [P0, 9], F32)
    din("wc_k0", [P0, 9], F32)
    din("wc_m1", [P0, 9], F32)
    din("wc_v0", [96, 9], F32)
    din("wc_v1", [96, 9], F32)
    din("ident", [P0, P0], BF16)
    t["out"] = nc.dram_tensor("out", [DIM, N], F32, kind="ExternalOutput").ap()

    with tile.TileContext(nc) as tc:
        _emit(tc, t)
    nc.compile()
    return nc


def _diag_pack(w, p):
    """w [p, 9] -> [p, 9, p] with m[c, t, c] = w[c, t]."""
    m = np.zeros((p, 9, p), np.float32)
    i = np.arange(p)
    m[i, :, i] = w
    return m.astype(BF)


def _prep_core(inp, c):
    b, br = divmod(c, 2)
    x = np.asarray(inp["x"], np.float32)
    cx = np.asarray(inp["ctx"], np.float32)
    if br == 0:  # ctx branch -> ctx_out[b]: q from x, k/v from ctx
        A, Bm = x[b], cx[b]
        wqkv_a, dw_a = inp["x_qkv_w"], inp["x_dw_w"]
        wqkv_b, dw_b = inp["ctx_qkv_w"], inp["ctx_dw_w"]
        po = inp["ctx_po_w"]
    else:        # x branch -> x_out[b]: q from ctx, k/v from x
        A, Bm = cx[b], x[b]
        wqkv_a, dw_a = inp["ctx_qkv_w"], inp["ctx_dw_w"]
        wqkv_b, dw_b = inp["x_qkv_w"], inp["x_dw_w"]
        po = inp["x_po_w"]
    wqkv_a = np.asarray(wqkv_a, np.float32)[:, :, 0, 0]
    wqkv_b = np.asarray(wqkv_b, np.float32)[:, :, 0, 0]
    dw_a = np.asarray(dw_a, np.float32)[:, 0].reshape(3 * DIM, 9)
    dw_b = np.asarray(dw_b, np.float32)[:, 0].reshape(3 * DIM, 9)
    po = np.asarray(po, np.float32)[:, :, 0, 0]
    temp = np.asarray(inp["temperature"], np.float32).reshape(NH)

    dq = dw_a[0:DIM]
    dk = dw_b[DIM:2 * DIM]
    dv = dw_b[2 * DIM:3 * DIM]
    m1 = np.zeros((P0, 9, P0), np.float32)
    i64 = np.arange(P1)
    m1[i64, :, i64] = dq[P0:DIM]
    m1[i64 + P1, :, i64 + P1] = dk[P0:DIM]

    wc_m1 = np.zeros((P0, 9), np.float32)
    wc_m1[0:P1] = dq[P0:DIM]
    wc_m1[P1:P0] = dk[P0:DIM]

    return {
        "wc_q0": dq[0:P0].astype(np.float32),
        "wc_k0": dk[0:P0].astype(np.float32),
        "wc_m1": wc_m1,
        "wc_v0": dv[0:96].astype(np.float32),
        "wc_v1": dv[96:DIM].astype(np.float32),
        "xa": np.ascontiguousarray(A.reshape(DIM, N)).astype(BF),
        "xb": np.ascontiguousarray(Bm.reshape(DIM, N)).astype(BF),
        "wq": np.ascontiguousarray(wqkv_a[0:DIM].T).astype(BF),
        "wk": np.ascontiguousarray(wqkv_b[DIM:2 * DIM].T).astype(BF),
        "wv": np.ascontiguousarray(wqkv_b[2 * DIM:3 * DIM].T).astype(BF),
        "wpo": np.ascontiguousarray(po.T).astype(BF),
        "dq0": _diag_pack(dq[0:P0], P0),
        "dk0": _diag_pack(dk[0:P0], P0),
        "dqk1": m1.astype(BF),
        "dv0": _diag_pack(dv[0:96], 96),
        "dv1": _diag_pack(dv[96:DIM], 96),
        "tsc": np.repeat(temp, HD)[:, None].astype(np.float32),
        "ident": np.eye(P0, dtype=np.float32).astype(BF),
    }


_BUILT = None
LAST_RESULTS = None


def kernel(**inputs):
    global _BUILT, LAST_RESULTS
    if _BUILT is None:
        _BUILT = _build()
    in_maps = [_prep_core(inputs, c) for c in range(8)]
    res = run_bass_kernel_spmd(
        _BUILT, in_maps, list(range(8)),
        trace=bool(int(os.environ.get("KTRACE", "0"))),
    )
    LAST_RESULTS = res
    outs = [np.asarray(r["out"], np.float32) for r in res.results]
    x_out = np.stack([outs[2 * b + 1].reshape(DIM, H, W) for b in range(4)])
    ctx_out = np.stack([outs[2 * b].reshape(DIM, H, W) for b in range(4)])
    return (x_out, ctx_out)
